# revision 5
# baseline (speedup 1.0000x reference)
"""ConvCrossAttention Trainium2 kernel — self-contained.

Problem (B=4, C_in=C_out=256, H=W=64, N=4096):
  q = conv1x1(x1, Wq, bq); k = conv1x1(x2, Wk, bk); v = conv1x1(x2, Wv, bv)
  out = softmax(q^T k / sqrt(C)) @ v^T, back in conv layout [B, C, H, W].

Sharding: data-parallel over (batch, query-half) -> 8 NeuronCores.
Core c handles batch c//2, query rows (c%2)*2048 : (c%2+1)*2048, with the
full 4096-key context for that batch. No collectives.

Per-core program (everything SBUF-resident):
  Phase A (streamed behind the input DMA): per 512-col x2 chunk j,
  project K and V^T; attention tiles of query-chunk 0 are interleaved one
  x2-chunk behind so the PE never waits on DMA. DMA triggers are merged
  (both 128-row halves per trigger) and split across the Sync queue
  (x1/x2/out) and the Activation + Pool queues (weights / biases) so the
  first K-projection starts ~3 us in.
  Phase B: query chunks 1..3, flash-style: S^T = K^T Q (PE), P = exp(S/16)
  (ACT, f32r out; no max-subtraction needed, |scores| < ~7), PV
  accumulated in PSUM (PE), P-sums split Pool/DVE. Each chunk's softmax
  tail (denominator matmul -> fast reciprocal -> broadcast matmul ->
  normalize + bias) is deferred INTO the next chunk's S stream (after
  tiles 1 and 5) so the in-order PE queue never stalls on the DVE chain.
  The final chunk's tail splits the two output halves across DVE and Pool.

All matmul operands are float32r (PE fast path, 1 cycle/row at >=256-wide
moving dim). Softmax denominators use reciprocal_approx_fast (~18-bit,
5x faster than InstReciprocal); inputs are sums of positive exps so the
undefined edge cases (0/denorm/inf) cannot occur.
"""

import sys

if "/opt/trn_rl_repo" not in sys.path:
    sys.path.insert(0, "/opt/trn_rl_repo")

from contextlib import ExitStack

import numpy as np

import concourse.bass as bass  # noqa: F401
import concourse.mybir as mybir
import concourse.tile as tile
from concourse import bacc
from concourse.bass_utils import run_bass_kernel_spmd

F32 = mybir.dt.float32
F32R = mybir.dt.float32r

B, C, H, W = 4, 256, 64, 64
N = H * W  # 4096
NQ = 2048  # queries per core (half a batch)
NK = 4096  # full key context
CHUNK = 512
NQ_CHUNKS = NQ // CHUNK
NK_TILES = NK // 128
XCHUNK = 512  # x2 DMA/projection chunk width
NJ = NK // XCHUNK  # 8 phase-A groups
SCALE = 1.0 / 16.0  # C ** -0.5
PIPE = 2  # PV matmuls trail S matmuls by this many nk tiles


def build_nc():
    MM = F32R
    nc = bacc.Bacc(None, debug=False)

    x1 = nc.dram_tensor("x1c", [C, NQ], MM, kind="ExternalInput")
    x2 = nc.dram_tensor("x2c", [C, NK], MM, kind="ExternalInput")
    wq = nc.dram_tensor("wqT", [C, C], MM, kind="ExternalInput")
    wk = nc.dram_tensor("wkT", [C, C], MM, kind="ExternalInput")
    wv = nc.dram_tensor("wvT", [C, C], MM, kind="ExternalInput")
    bq = nc.dram_tensor("bq", [C, 1], F32, kind="ExternalInput")
    bk = nc.dram_tensor("bk", [C, 1], F32, kind="ExternalInput")
    bv = nc.dram_tensor("bv", [C, 1], F32, kind="ExternalInput")
    out = nc.dram_tensor("out", [C, NQ], F32, kind="ExternalOutput")

    def split_h(ap):  # DRAM [256, w] -> [128, 2, w] (partition-first)
        return ap.rearrange("(h p) w -> p h w", p=128)

    with tile.TileContext(nc) as tc, ExitStack() as ctx:
        big = ctx.enter_context(tc.tile_pool(name="big", bufs=1))
        small = ctx.enter_context(tc.tile_pool(name="small", bufs=1))
        ppool = ctx.enter_context(tc.tile_pool(name="p", bufs=4))
        opool = ctx.enter_context(tc.tile_pool(name="o", bufs=2))
        dpool = ctx.enter_context(tc.tile_pool(name="d", bufs=2))
        spsum = ctx.enter_context(tc.tile_pool(name="spsum", bufs=2, space="PSUM"))
        apsum = ctx.enter_context(tc.tile_pool(name="apsum", bufs=4, space="PSUM"))
        dpsum = ctx.enter_context(tc.tile_pool(name="dpsum", bufs=1, space="PSUM"))

        # --- SBUF residents ---
        wq_sb = small.tile([128, 2, C], MM, tag="wq")
        wk_sb = small.tile([128, 2, C], MM, tag="wk")
        wv_sb = small.tile([128, 2, C], MM, tag="wv")
        bq_sb = small.tile([128, 2, 1], F32, tag="bq")
        bk_sb = small.tile([128, 2, 1], F32, tag="bk")
        bv_sb = small.tile([128, 2, 1], F32, tag="bv")
        x1_sb = big.tile([128, 2, NQ], MM, tag="x1")
        x2_sb = big.tile([128, 2, NK], MM, tag="x2")
        q_sb = big.tile([128, 2, NQ], MM, tag="q")
        k_sb = big.tile([128, 2, NK], MM, tag="k")
        v_sb = big.tile([128, NK_TILES, C], MM, tag="v")

        # --- DMA triggers, earliest. Sync queue: x2/x1 (the critical
        # stream); Activation queue: weights; Pool queue: biases. Each
        # trigger moves both 128-row halves (merged descriptor). ---
        nc.sync.dma_start(out=x2_sb[:, :, 0:XCHUNK], in_=split_h(x2[:, 0:XCHUNK]))
        nc.sync.dma_start(out=x1_sb[:, :, 0:CHUNK], in_=split_h(x1[:, 0:CHUNK]))
        for j in range(1, NJ):
            xs_ = slice(j * XCHUNK, (j + 1) * XCHUNK)
            nc.sync.dma_start(out=x2_sb[:, :, xs_], in_=split_h(x2[:, xs_]))
        nc.sync.dma_start(out=x1_sb[:, :, CHUNK:NQ], in_=split_h(x1[:, CHUNK:NQ]))

        nc.scalar.dma_start(out=wk_sb[:], in_=split_h(wk[:, :]))
        nc.scalar.dma_start(out=wq_sb[:], in_=split_h(wq[:, :]))
        nc.scalar.dma_start(out=wv_sb[:], in_=split_h(wv[:, :]))
        nc.gpsimd.dma_start(out=bk_sb[:], in_=split_h(bk[:, :]))
        nc.gpsimd.dma_start(out=bq_sb[:], in_=split_h(bq[:, :]))
        nc.gpsimd.dma_start(out=bv_sb[:], in_=split_h(bv[:, :]))

        ones_col_f32 = small.tile([128, 1], F32, tag="ones_col_f32")
        nc.vector.memset(ones_col_f32[:], 1.0)
        ones_col = small.tile([128, 1], MM, tag="ones_col")
        nc.vector.tensor_copy(ones_col[:], ones_col_f32[:])
        ones_row_f32 = small.tile([1, 128], F32, tag="ones_row_f32")
        nc.vector.memset(ones_row_f32[:], 1.0)
        ones_row = small.tile([1, 128], MM, tag="ones_row")
        nc.vector.tensor_copy(ones_row[:], ones_row_f32[:])

        # --- projection helpers ---
        def kproj(j):
            cs = slice(j * XCHUNK, (j + 1) * XCHUNK)
            for ct in range(2):
                kp = spsum.tile([128, XCHUNK], F32, tag="s", name="kp")
                cts = slice(ct * 128, (ct + 1) * 128)
                nc.tensor.matmul(kp[:], wk_sb[:, 0, cts], x2_sb[:, 0, cs], start=True, stop=False)
                nc.tensor.matmul(kp[:], wk_sb[:, 1, cts], x2_sb[:, 1, cs], start=False, stop=True)
                nc.vector.tensor_scalar_add(k_sb[:, ct, cs], kp[:], bk_sb[:, ct, :])

        def vproj(j):
            for t in range(j * (XCHUNK // 128), (j + 1) * (XCHUNK // 128)):
                ts = slice(t * 128, (t + 1) * 128)
                vp = spsum.tile([128, C], F32, tag="s", name="vp")
                nc.tensor.matmul(vp[:], x2_sb[:, 0, ts], wv_sb[:, 0, :], start=True, stop=False)
                nc.tensor.matmul(vp[:], x2_sb[:, 1, ts], wv_sb[:, 1, :], start=False, stop=True)
                nc.scalar.copy(v_sb[:, t, :], vp[:])

        def qproj(c0):
            cs = slice(c0 * CHUNK, (c0 + 1) * CHUNK)
            for ct in range(2):
                qp = spsum.tile([128, CHUNK], F32, tag="s", name="qp")
                cts = slice(ct * 128, (ct + 1) * 128)
                nc.tensor.matmul(qp[:], wq_sb[:, 0, cts], x1_sb[:, 0, cs], start=True, stop=False)
                nc.tensor.matmul(qp[:], wq_sb[:, 1, cts], x1_sb[:, 1, cs], start=False, stop=True)
                nc.vector.tensor_scalar_add(q_sb[:, ct, cs], qp[:], bq_sb[:, ct, :])

        # --- attention chunk state ---
        class ChunkState:
            def __init__(self, c0):
                self.c0 = c0
                self.cs = slice(c0 * CHUNK, (c0 + 1) * CHUNK)
                self.acc0 = apsum.tile([128, CHUNK], F32, tag="acc", name="acc0")
                self.acc1 = apsum.tile([128, CHUNK], F32, tag="acc", name="acc1")
                # P-sum split across Pool (even tiles) and DVE (odd) so
                # neither engine's serial accumulation chain gates the PE.
                self.psum_p = dpool.tile([128, CHUNK], F32, tag="psum_p", name="psum_p")
                self.psum_d = dpool.tile([128, CHUNK], F32, tag="psum_d", name="psum_d")
                self.p_tiles = {}

        def s_tile(st, t):
            ts = slice(t * 128, (t + 1) * 128)
            sp = spsum.tile([128, CHUNK], F32, tag="s", name="sp")
            nc.tensor.matmul(sp[:], k_sb[:, 0, ts], q_sb[:, 0, st.cs], start=True, stop=False)
            nc.tensor.matmul(sp[:], k_sb[:, 1, ts], q_sb[:, 1, st.cs], start=False, stop=True)
            p = ppool.tile([128, CHUNK], MM, tag="p", name="p")
            nc.scalar.activation(p[:], sp[:], mybir.ActivationFunctionType.Exp, scale=SCALE)
            st.p_tiles[t] = p

        def emit_pv(st, t):
            first, last = t == 0, t == NK_TILES - 1
            p = st.p_tiles.pop(t)
            nc.tensor.matmul(st.acc0[:], v_sb[:, t, 0:128], p[:], start=first, stop=last)
            nc.tensor.matmul(st.acc1[:], v_sb[:, t, 128:256], p[:], start=first, stop=last)
            eng, acc_ps = (nc.gpsimd, st.psum_p) if t % 2 == 0 else (nc.vector, st.psum_d)
            if t < 2:
                eng.tensor_copy(acc_ps[:], p[:].bitcast(F32))
            else:
                eng.tensor_add(acc_ps[:], acc_ps[:], p[:].bitcast(F32))

        # --- softmax tails. tail_a: denominator + reciprocal. tail_b:
        # broadcast + normalize + bias + out DMA. Both run for chunk c
        # while chunk c+1's S/PV stream keeps the PE busy; `final` splits
        # the output halves across DVE and Pool to shorten the exposed
        # end-of-kernel chain. ---
        def tail_a(st):
            acc_r = dpool.tile([128, CHUNK], MM, tag="acc_r", name="acc_r")
            nc.vector.tensor_add(acc_r[:], st.psum_p[:], st.psum_d[:])
            den = dpsum.tile([1, CHUNK], F32, tag="den", name="den")
            nc.tensor.matmul(den[:], ones_col[:], acc_r[:], start=True, stop=True)
            recip_f32 = dpool.tile([1, CHUNK], F32, tag="recip_f32", name="recip_f32")
            nc.vector.reciprocal_approx_fast(out=recip_f32[:], in_=den[:])
            recip = dpool.tile([1, CHUNK], MM, tag="recip", name="recip")
            nc.vector.tensor_copy(recip[:], recip_f32[:])
            st.recip = recip

        def tail_b(st, final=False):
            bcast = dpsum.tile([128, CHUNK], F32, tag="bcast", name="bcast")
            nc.tensor.matmul(bcast[:], ones_row[:], st.recip[:], start=True, stop=True)
            bcast_sb = opool.tile([128, CHUNK], F32, tag="bcast_sb", name="bcast_sb")
            nc.vector.tensor_copy(bcast_sb[:], bcast[:])
            o2 = opool.tile([128, 2, CHUNK], F32, tag="o2", name="o2")
            if final:
                # split halves across DVE (ct0, PSUM-capable) and Pool (ct1,
                # via an early ACT-engine PSUM->SBUF stage) so the exposed
                # end-of-kernel chain is half as long
                acc1_sb = opool.tile([128, CHUNK], F32, tag="acc1_sb", name="acc1_sb")
                nc.scalar.copy(acc1_sb[:], st.acc1[:])
                nc.vector.tensor_mul(o2[:, 0, :], st.acc0[:], bcast_sb[:])
                nc.vector.tensor_scalar_add(o2[:, 0, :], o2[:, 0, :], bv_sb[:, 0, :])
                nc.sync.dma_start(
                    out=split_h(out[:, st.cs])[:, 0:1, :], in_=o2[:, 0:1, :]
                )
                nc.gpsimd.tensor_mul(o2[:, 1, :], acc1_sb[:], bcast_sb[:])
                nc.gpsimd.tensor_scalar_add(o2[:, 1, :], o2[:, 1, :], bv_sb[:, 1, :])
                nc.sync.dma_start(
                    out=split_h(out[:, st.cs])[:, 1:2, :], in_=o2[:, 1:2, :]
                )
            else:
                for ct, acc in ((0, st.acc0), (1, st.acc1)):
                    nc.vector.tensor_mul(o2[:, ct, :], acc[:], bcast_sb[:])
                    nc.vector.tensor_scalar_add(o2[:, ct, :], o2[:, ct, :], bv_sb[:, ct, :])
                nc.sync.dma_start(out=split_h(out[:, st.cs]), in_=o2[:])

        # ================= program =================
        # Phase A: stream projections behind DMA; interleave chunk-0
        # attention one x2-chunk behind the projections.
        kproj(0)
        qproj(0)
        vproj(0)
        st = ChunkState(0)
        for j in range(1, NJ):
            for t in range((j - 1) * 4, j * 4):
                s_tile(st, t)
                if t >= PIPE:
                    emit_pv(st, t - PIPE)
            kproj(j)
            vproj(j)
        for t in range((NJ - 1) * 4, NK_TILES):
            s_tile(st, t)
            if t >= PIPE:
                emit_pv(st, t - PIPE)
        for t in range(NK_TILES - PIPE, NK_TILES):
            emit_pv(st, t)
        for c0 in range(1, NQ_CHUNKS):
            qproj(c0)

        # Phase B: chunks 1..3; previous chunk's tail is woven into this
        # chunk's S stream (tail_a after tile 1, tail_b after tile 5).
        prev = st
        for c0 in range(1, NQ_CHUNKS):
            st = ChunkState(c0)
            for t in range(NK_TILES):
                s_tile(st, t)
                if t == 1 and prev is not None:
                    tail_a(prev)
                if t == 5 and prev is not None:
                    tail_b(prev)
                    prev = None
                if t >= PIPE:
                    emit_pv(st, t - PIPE)
            for t in range(NK_TILES - PIPE, NK_TILES):
                emit_pv(st, t)
            prev = st

        # final chunk's tail is exposed: shortest possible chain
        tail_a(prev)
        tail_b(prev, final=True)

    nc.compile()
    return nc


def core_inputs(inputs, core):
    """Slice full-problem inputs for one core (numpy)."""
    b, h = core // 2, core % 2
    x1r = np.asarray(inputs["x1"], dtype=np.float32).reshape(B, C, N)
    x2r = np.asarray(inputs["x2"], dtype=np.float32).reshape(B, C, N)
    return {
        "x1c": np.ascontiguousarray(x1r[b][:, h * NQ : (h + 1) * NQ]),
        "x2c": np.ascontiguousarray(x2r[b]),
        "wqT": np.ascontiguousarray(np.asarray(inputs["Wq"], dtype=np.float32).T),
        "wkT": np.ascontiguousarray(np.asarray(inputs["Wk"], dtype=np.float32).T),
        "wvT": np.ascontiguousarray(np.asarray(inputs["Wv"], dtype=np.float32).T),
        "bq": np.asarray(inputs["bq"], dtype=np.float32).reshape(C, 1).copy(),
        "bk": np.asarray(inputs["bk"], dtype=np.float32).reshape(C, 1).copy(),
        "bv": np.asarray(inputs["bv"], dtype=np.float32).reshape(C, 1).copy(),
    }


_NC_CACHE = {}


def get_nc():
    if "nc" not in _NC_CACHE:
        _NC_CACHE["nc"] = build_nc()
    return _NC_CACHE["nc"]


def kernel(**inputs) -> np.ndarray:
    """Full-problem entry point: full inputs in, full [4,256,64,64] f32 out."""
    nc = get_nc()
    in_maps = [core_inputs(inputs, core) for core in range(8)]
    res = run_bass_kernel_spmd(nc, in_maps, list(range(8)))
    full = np.zeros((B, C, N), np.float32)
    for core in range(8):
        b, h = core // 2, core % 2
        full[b][:, h * NQ : (h + 1) * NQ] = res.results[core]["out"]
    return full.reshape(B, C, H, W)


# revision 12
# speedup vs baseline: 1.0080x; 1.0080x over previous
"""ConvCrossAttention Trainium2 kernel — self-contained.

Problem (B=4, C_in=C_out=256, H=W=64, N=4096):
  q = conv1x1(x1, Wq, bq); k = conv1x1(x2, Wk, bk); v = conv1x1(x2, Wv, bv)
  out = softmax(q^T k / sqrt(C)) @ v^T, back in conv layout [B, C, H, W].

Sharding: data-parallel over (batch, query-half) -> 8 NeuronCores.
Core c handles batch c//2, query rows (c%2)*2048 : (c%2+1)*2048, with the
full 4096-key context for that batch. No collectives.

Per-core program (everything SBUF-resident):
  Phase A (streamed behind the input DMA): per 512-col x2 chunk j,
  project K and V^T; attention tiles of query-chunk 0 are interleaved one
  x2-chunk behind so the PE never waits on DMA. DMA triggers are merged
  (both 128-row halves per trigger) and split across the Sync queue
  (x1/x2/out) and the Activation + Pool queues (weights / biases) so the
  first K-projection starts ~3 us in.
  Phase B: query chunks 1..3, flash-style: S^T = K^T Q (PE), P = exp(S/16)
  (ACT, f32r out; no max-subtraction needed, |scores| < ~7), PV
  accumulated in PSUM (PE), P-sums split Pool/DVE. Each chunk's softmax
  tail (denominator matmul -> fast reciprocal -> broadcast matmul ->
  normalize + bias) is deferred INTO the next chunk's S stream (after
  tiles 1 and 5) so the in-order PE queue never stalls on the DVE chain.
  The final chunk's tail splits the two output halves across DVE and Pool.

All matmul operands are float32r (PE fast path, 1 cycle/row at >=256-wide
moving dim). Softmax denominators use reciprocal_approx_fast (~18-bit,
5x faster than InstReciprocal); inputs are sums of positive exps so the
undefined edge cases (0/denorm/inf) cannot occur.
"""

import sys

if "/opt/trn_rl_repo" not in sys.path:
    sys.path.insert(0, "/opt/trn_rl_repo")

from contextlib import ExitStack

import numpy as np

import concourse.bass as bass  # noqa: F401
import concourse.mybir as mybir
import concourse.tile as tile
from concourse import bacc
from concourse.bass_utils import run_bass_kernel_spmd

F32 = mybir.dt.float32
F32R = mybir.dt.float32r

B, C, H, W = 4, 256, 64, 64
N = H * W  # 4096
NQ = 2048  # queries per core (half a batch)
NK = 4096  # full key context
CHUNK = 512
NQ_CHUNKS = NQ // CHUNK
NK_TILES = NK // 128
XCHUNK = 512  # x2 DMA/projection chunk width
NJ = NK // XCHUNK  # 8 phase-A groups
SCALE = 1.0 / 16.0  # C ** -0.5
PIPE = 2  # PV matmuls trail S matmuls by this many nk tiles


def build_nc():
    MM = F32R
    nc = bacc.Bacc(None, debug=False)

    x1 = nc.dram_tensor("x1c", [C, NQ], MM, kind="ExternalInput")
    x2 = nc.dram_tensor("x2c", [C, NK], MM, kind="ExternalInput")
    wq = nc.dram_tensor("wqT", [C, C], MM, kind="ExternalInput")
    wk = nc.dram_tensor("wkT", [C, C], MM, kind="ExternalInput")
    wv = nc.dram_tensor("wvT", [C, C], MM, kind="ExternalInput")
    bq = nc.dram_tensor("bq", [C, 1], F32, kind="ExternalInput")
    bk = nc.dram_tensor("bk", [C, 1], F32, kind="ExternalInput")
    bv = nc.dram_tensor("bv", [C, 1], F32, kind="ExternalInput")
    out = nc.dram_tensor("out", [C, NQ], F32, kind="ExternalOutput")

    def split_h(ap):  # DRAM [256, w] -> [128, 2, w] (partition-first)
        return ap.rearrange("(h p) w -> p h w", p=128)

    with tile.TileContext(nc) as tc, ExitStack() as ctx:
        big = ctx.enter_context(tc.tile_pool(name="big", bufs=1))
        small = ctx.enter_context(tc.tile_pool(name="small", bufs=1))
        ppool = ctx.enter_context(tc.tile_pool(name="p", bufs=6))
        opool = ctx.enter_context(tc.tile_pool(name="o", bufs=2))
        dpool = ctx.enter_context(tc.tile_pool(name="d", bufs=2))
        spsum = ctx.enter_context(tc.tile_pool(name="spsum", bufs=2, space="PSUM"))
        apsum = ctx.enter_context(tc.tile_pool(name="apsum", bufs=4, space="PSUM"))
        dpsum = ctx.enter_context(tc.tile_pool(name="dpsum", bufs=1, space="PSUM"))

        # --- SBUF residents ---
        wq_sb = small.tile([128, 2, C], MM, tag="wq")
        wk_sb = small.tile([128, 2, C], MM, tag="wk")
        wv_sb = small.tile([128, 2, C], MM, tag="wv")
        bq_sb = small.tile([128, 2, 1], F32, tag="bq")
        bk_sb = small.tile([128, 2, 1], F32, tag="bk")
        bv_sb = small.tile([128, 2, 1], F32, tag="bv")
        x1_sb = big.tile([128, 2, NQ], MM, tag="x1")
        x2_sb = big.tile([128, 2, NK], MM, tag="x2")
        q_sb = big.tile([128, 2, NQ], MM, tag="q")
        k_sb = big.tile([128, 2, NK], MM, tag="k")
        v_sb = big.tile([128, NK_TILES, C], MM, tag="v")

        # --- DMA triggers, earliest; ordered by first consumption. Sync
        # queue carries the critical stream (weights + x-data) since its
        # preamble clears first; Activation queue (blocked ~1.3us longer by
        # the exp table load) carries the biases, needed slightly later.
        # Each trigger moves both 128-row halves (merged descriptor). ---
        nc.sync.dma_start(out=wk_sb[:], in_=split_h(wk[:, :]))
        nc.sync.dma_start(out=x2_sb[:, :, 0:XCHUNK], in_=split_h(x2[:, 0:XCHUNK]))
        nc.sync.dma_start(out=x1_sb[:, :, 0:CHUNK], in_=split_h(x1[:, 0:CHUNK]))
        nc.sync.dma_start(out=wq_sb[:], in_=split_h(wq[:, :]))
        nc.sync.dma_start(out=wv_sb[:], in_=split_h(wv[:, :]))
        for j in range(1, NJ):
            xs_ = slice(j * XCHUNK, (j + 1) * XCHUNK)
            nc.sync.dma_start(out=x2_sb[:, :, xs_], in_=split_h(x2[:, xs_]))
        nc.sync.dma_start(out=x1_sb[:, :, CHUNK:NQ], in_=split_h(x1[:, CHUNK:NQ]))

        nc.scalar.dma_start(out=bk_sb[:], in_=split_h(bk[:, :]))
        nc.scalar.dma_start(out=bq_sb[:], in_=split_h(bq[:, :]))
        nc.scalar.dma_start(out=bv_sb[:], in_=split_h(bv[:, :]))
        # bv as a [1, 2, 128] f32r row for the bias-fold matmul of the
        # final chunk (bias enters as bv (x) den before normalization)
        bv_row = small.tile([1, 2, 128], MM, tag="bv_row")
        nc.scalar.dma_start(
            out=bv_row[:], in_=bv[:, :].rearrange("(h p) o -> o h p", p=128).bitcast(F32R)
        )

        ones_col_f32 = small.tile([128, 1], F32, tag="ones_col_f32")
        nc.vector.memset(ones_col_f32[:], 1.0)
        ones_col = small.tile([128, 1], MM, tag="ones_col")
        nc.vector.tensor_copy(ones_col[:], ones_col_f32[:])
        ones_row_f32 = small.tile([1, 128], F32, tag="ones_row_f32")
        nc.vector.memset(ones_row_f32[:], 1.0)
        ones_row = small.tile([1, 128], MM, tag="ones_row")
        nc.vector.tensor_copy(ones_row[:], ones_row_f32[:])

        # --- projection helpers ---
        def kproj(j):
            cs = slice(j * XCHUNK, (j + 1) * XCHUNK)
            for ct in range(2):
                kp = spsum.tile([128, XCHUNK], F32, tag="s", name="kp")
                cts = slice(ct * 128, (ct + 1) * 128)
                nc.tensor.matmul(kp[:], wk_sb[:, 0, cts], x2_sb[:, 0, cs], start=True, stop=False)
                nc.tensor.matmul(kp[:], wk_sb[:, 1, cts], x2_sb[:, 1, cs], start=False, stop=True)
                nc.vector.tensor_scalar_add(k_sb[:, ct, cs], kp[:], bk_sb[:, ct, :])

        def vproj(j):
            for t in range(j * (XCHUNK // 128), (j + 1) * (XCHUNK // 128)):
                ts = slice(t * 128, (t + 1) * 128)
                vp = spsum.tile([128, C], F32, tag="s", name="vp")
                nc.tensor.matmul(vp[:], x2_sb[:, 0, ts], wv_sb[:, 0, :], start=True, stop=False)
                nc.tensor.matmul(vp[:], x2_sb[:, 1, ts], wv_sb[:, 1, :], start=False, stop=True)
                nc.scalar.copy(v_sb[:, t, :], vp[:])

        def qproj(c0):
            cs = slice(c0 * CHUNK, (c0 + 1) * CHUNK)
            for ct in range(2):
                qp = spsum.tile([128, CHUNK], F32, tag="s", name="qp")
                cts = slice(ct * 128, (ct + 1) * 128)
                nc.tensor.matmul(qp[:], wq_sb[:, 0, cts], x1_sb[:, 0, cs], start=True, stop=False)
                nc.tensor.matmul(qp[:], wq_sb[:, 1, cts], x1_sb[:, 1, cs], start=False, stop=True)
                nc.vector.tensor_scalar_add(q_sb[:, ct, cs], qp[:], bq_sb[:, ct, :])

        # --- attention chunk state ---
        class ChunkState:
            def __init__(self, c0):
                self.c0 = c0
                self.cs = slice(c0 * CHUNK, (c0 + 1) * CHUNK)
                self.acc0 = apsum.tile([128, CHUNK], F32, tag="acc", name="acc0")
                self.acc1 = apsum.tile([128, CHUNK], F32, tag="acc", name="acc1")
                # P-sum split across Pool (even tiles) and DVE (odd) so
                # neither engine's serial accumulation chain gates the PE.
                self.psum_p = dpool.tile([128, CHUNK], F32, tag="psum_p", name="psum_p")
                self.psum_d = dpool.tile([128, CHUNK], F32, tag="psum_d", name="psum_d")
                self.p_tiles = {}

        def s_tile(st, t):
            ts = slice(t * 128, (t + 1) * 128)
            sp = spsum.tile([128, CHUNK], F32, tag="s", name="sp")
            nc.tensor.matmul(sp[:], k_sb[:, 0, ts], q_sb[:, 0, st.cs], start=True, stop=False)
            nc.tensor.matmul(sp[:], k_sb[:, 1, ts], q_sb[:, 1, st.cs], start=False, stop=True)
            p = ppool.tile([128, CHUNK], MM, tag="p", name="p")
            nc.scalar.activation(p[:], sp[:], mybir.ActivationFunctionType.Exp, scale=SCALE)
            st.p_tiles[t] = p

        def emit_pv(st, t, final=False):
            first = t == 0
            last = t == NK_TILES - 1 and not final  # final: bias matmul closes
            p = st.p_tiles.pop(t)
            nc.tensor.matmul(st.acc0[:], v_sb[:, t, 0:128], p[:], start=first, stop=last)
            nc.tensor.matmul(st.acc1[:], v_sb[:, t, 128:256], p[:], start=first, stop=last)
            if t == NK_TILES - 1:
                # last tile's P joins via the tree-balanced combine below
                st.p31 = p
                return
            eng, acc_ps = (nc.gpsimd, st.psum_p) if t % 2 == 0 else (nc.vector, st.psum_d)
            if t < 2:
                eng.tensor_copy(acc_ps[:], p[:].bitcast(F32))
            else:
                eng.tensor_add(acc_ps[:], acc_ps[:], p[:].bitcast(F32))
            if t == NK_TILES - 2:
                # evens(0..30) + odds(1..29) combine, off the critical path
                st.comb = dpool.tile([128, CHUNK], F32, tag="comb", name="comb")
                nc.gpsimd.tensor_add(st.comb[:], st.psum_p[:], st.psum_d[:])

        def flush_chunk(st, final=False):
            for t in range(NK_TILES - PIPE, NK_TILES):
                emit_pv(st, t, final=final)
            # P total = comb + p31; one short DVE link after the last exp
            st.acc_r = dpool.tile([128, CHUNK], MM, tag="acc_r", name="acc_r")
            nc.vector.tensor_add(st.acc_r[:], st.comb[:], st.p31[:].bitcast(F32))

        # --- softmax tails. tail_a: denominator + reciprocal. tail_b:
        # broadcast + normalize + bias + out DMA. Both run for chunk c
        # while chunk c+1's S/PV stream keeps the PE busy; `final` splits
        # the output halves across DVE and Pool to shorten the exposed
        # end-of-kernel chain. ---
        def tail_a(st, final=False):
            den = dpsum.tile([1, CHUNK], F32, tag="den", name="den")
            nc.tensor.matmul(den[:], ones_col[:], st.acc_r[:], start=True, stop=True)
            if final:
                # fold bias in before normalization: acc_ct += bv_ct (x) den,
                # so the exposed end chain needs no per-half bias add
                den_sb = dpool.tile([1, CHUNK], MM, tag="den_sb", name="den_sb")
                nc.scalar.copy(den_sb[:], den[:])
                nc.tensor.matmul(st.acc0[:], bv_row[:, 0, :], den_sb[:], start=False, stop=True)
                nc.tensor.matmul(st.acc1[:], bv_row[:, 1, :], den_sb[:], start=False, stop=True)
            recip_f32 = dpool.tile([1, CHUNK], F32, tag="recip_f32", name="recip_f32")
            nc.vector.reciprocal_approx_fast(out=recip_f32[:], in_=den[:])
            recip = dpool.tile([1, CHUNK], MM, tag="recip", name="recip")
            nc.vector.tensor_copy(recip[:], recip_f32[:])
            st.recip = recip

        def tail_b(st, final=False):
            bcast = dpsum.tile([128, CHUNK], F32, tag="bcast", name="bcast")
            nc.tensor.matmul(bcast[:], ones_row[:], st.recip[:], start=True, stop=True)
            bcast_sb = opool.tile([128, CHUNK], F32, tag="bcast_sb", name="bcast_sb")
            nc.vector.tensor_copy(bcast_sb[:], bcast[:])
            if final:
                # bias already folded; separate tiles per half so neither
                # DMA waits on the other half's writer
                o0 = opool.tile([128, CHUNK], F32, tag="o_f0", name="o_f0")
                nc.vector.tensor_mul(o0[:], st.acc0[:], bcast_sb[:])
                nc.sync.dma_start(
                    out=split_h(out[:, st.cs])[:, 0:1, :],
                    in_=o0[:].rearrange("p (o w) -> p o w", o=1),
                )
                o1 = opool.tile([128, CHUNK], F32, tag="o_f1", name="o_f1")
                nc.vector.tensor_mul(o1[:], st.acc1[:], bcast_sb[:])
                nc.sync.dma_start(
                    out=split_h(out[:, st.cs])[:, 1:2, :],
                    in_=o1[:].rearrange("p (o w) -> p o w", o=1),
                )
            else:
                o2 = opool.tile([128, 2, CHUNK], F32, tag="o2", name="o2")
                for ct, acc in ((0, st.acc0), (1, st.acc1)):
                    nc.vector.tensor_mul(o2[:, ct, :], acc[:], bcast_sb[:])
                    nc.vector.tensor_scalar_add(o2[:, ct, :], o2[:, ct, :], bv_sb[:, ct, :])
                nc.sync.dma_start(out=split_h(out[:, st.cs]), in_=o2[:])

        # ================= program =================
        # Phase A: stream projections behind DMA; interleave chunk-0
        # attention one x2-chunk behind the projections.
        kproj(0)
        qproj(0)
        vproj(0)
        st = ChunkState(0)
        for j in range(1, NJ):
            for t in range((j - 1) * 4, j * 4):
                s_tile(st, t)
                if t >= PIPE:
                    emit_pv(st, t - PIPE)
            kproj(j)
            vproj(j)
        for t in range((NJ - 1) * 4, NK_TILES):
            s_tile(st, t)
            if t >= PIPE:
                emit_pv(st, t - PIPE)
        flush_chunk(st)
        for c0 in range(1, NQ_CHUNKS):
            qproj(c0)

        # Phase B: chunks 1..3; previous chunk's tail is woven into this
        # chunk's S stream (tail_a after tile 1, tail_b after tile 5).
        prev = st
        for c0 in range(1, NQ_CHUNKS):
            st = ChunkState(c0)
            last = c0 == NQ_CHUNKS - 1
            for t in range(NK_TILES):
                s_tile(st, t)
                if t == 1 and prev is not None:
                    tail_a(prev)
                if t == 5 and prev is not None:
                    tail_b(prev)
                    prev = None
                if t >= PIPE and t - PIPE < NK_TILES - PIPE:
                    emit_pv(st, t - PIPE)
            flush_chunk(st, final=last)
            prev = st

        # final chunk's tail is exposed: shortest possible chain
        tail_a(prev, final=True)
        tail_b(prev, final=True)

    nc.compile()
    return nc


def core_inputs(inputs, core):
    """Slice full-problem inputs for one core (numpy)."""
    b, h = core // 2, core % 2
    x1r = np.asarray(inputs["x1"], dtype=np.float32).reshape(B, C, N)
    x2r = np.asarray(inputs["x2"], dtype=np.float32).reshape(B, C, N)
    return {
        "x1c": np.ascontiguousarray(x1r[b][:, h * NQ : (h + 1) * NQ]),
        "x2c": np.ascontiguousarray(x2r[b]),
        "wqT": np.ascontiguousarray(np.asarray(inputs["Wq"], dtype=np.float32).T),
        "wkT": np.ascontiguousarray(np.asarray(inputs["Wk"], dtype=np.float32).T),
        "wvT": np.ascontiguousarray(np.asarray(inputs["Wv"], dtype=np.float32).T),
        "bq": np.asarray(inputs["bq"], dtype=np.float32).reshape(C, 1).copy(),
        "bk": np.asarray(inputs["bk"], dtype=np.float32).reshape(C, 1).copy(),
        "bv": np.asarray(inputs["bv"], dtype=np.float32).reshape(C, 1).copy(),
    }


_NC_CACHE = {}


def get_nc():
    if "nc" not in _NC_CACHE:
        _NC_CACHE["nc"] = build_nc()
    return _NC_CACHE["nc"]


def kernel(**inputs) -> np.ndarray:
    """Full-problem entry point: full inputs in, full [4,256,64,64] f32 out."""
    nc = get_nc()
    in_maps = [core_inputs(inputs, core) for core in range(8)]
    res = run_bass_kernel_spmd(nc, in_maps, list(range(8)))
    full = np.zeros((B, C, N), np.float32)
    for core in range(8):
        b, h = core // 2, core % 2
        full[b][:, h * NQ : (h + 1) * NQ] = res.results[core]["out"]
    return full.reshape(B, C, H, W)


# revision 13
# speedup vs baseline: 1.0734x; 1.0649x over previous
"""ConvCrossAttention Trainium2 kernel — self-contained.

Problem (B=4, C_in=C_out=256, H=W=64, N=4096):
  q = conv1x1(x1, Wq, bq); k = conv1x1(x2, Wk, bk); v = conv1x1(x2, Wv, bv)
  out = softmax(q^T k / sqrt(C)) @ v^T, back in conv layout [B, C, H, W].

Sharding: data-parallel over (batch, query-half) -> 8 NeuronCores.
Core c handles batch c//2, query rows (c%2)*2048 : (c%2+1)*2048, with the
full 4096-key context for that batch. No collectives.

Per-core program (everything SBUF-resident):
  Phase A (streamed behind the input DMA): per 512-col x2 chunk j,
  project K and V^T; attention tiles of query-chunk 0 are interleaved one
  x2-chunk behind so the PE never waits on DMA. DMA triggers are merged
  (both 128-row halves per trigger) and split across the Sync queue
  (x1/x2/out) and the Activation + Pool queues (weights / biases) so the
  first K-projection starts ~3 us in.
  Phase B: query chunks 1..3, flash-style: S^T = K^T Q (PE), P = exp(S/16)
  (ACT, f32r out; no max-subtraction needed, |scores| < ~7), PV
  accumulated in PSUM (PE), P-sums split Pool/DVE. Each chunk's softmax
  tail (denominator matmul -> fast reciprocal -> broadcast matmul ->
  normalize + bias) is deferred INTO the next chunk's S stream (after
  tiles 1 and 5) so the in-order PE queue never stalls on the DVE chain.
  The final chunk's tail splits the two output halves across DVE and Pool.

All matmul operands are float32r (PE fast path, 1 cycle/row at >=256-wide
moving dim). Softmax denominators use reciprocal_approx_fast (~18-bit,
5x faster than InstReciprocal); inputs are sums of positive exps so the
undefined edge cases (0/denorm/inf) cannot occur.
"""

import sys

if "/opt/trn_rl_repo" not in sys.path:
    sys.path.insert(0, "/opt/trn_rl_repo")

from contextlib import ExitStack

import numpy as np

import concourse.bass as bass  # noqa: F401
import concourse.mybir as mybir
import concourse.tile as tile
from concourse import bacc
from concourse.bass_utils import run_bass_kernel_spmd

F32 = mybir.dt.float32
F32R = mybir.dt.float32r
F16 = mybir.dt.float16

B, C, H, W = 4, 256, 64, 64
N = H * W  # 4096
NQ = 2048  # queries per core (half a batch)
NK = 4096  # full key context
CHUNK = 512
NQ_CHUNKS = NQ // CHUNK
NK_TILES = NK // 128
XCHUNK = 512  # x2 DMA/projection chunk width
NJ = NK // XCHUNK  # 8 phase-A groups
SCALE = 1.0 / 16.0  # C ** -0.5
PIPE = 2  # PV matmuls trail S matmuls by this many nk tiles


def build_nc():
    MM = F32R
    nc = bacc.Bacc(None, debug=False)

    x1 = nc.dram_tensor("x1c", [C, NQ], MM, kind="ExternalInput")
    x2 = nc.dram_tensor("x2c", [C, NK], MM, kind="ExternalInput")
    wq = nc.dram_tensor("wqT", [C, C], MM, kind="ExternalInput")
    wk = nc.dram_tensor("wkT", [C, C], MM, kind="ExternalInput")
    wv = nc.dram_tensor("wvT", [C, C], MM, kind="ExternalInput")
    bq = nc.dram_tensor("bq", [C, 1], F32, kind="ExternalInput")
    bk = nc.dram_tensor("bk", [C, 1], F32, kind="ExternalInput")
    bv = nc.dram_tensor("bv", [C, 1], F32, kind="ExternalInput")
    out = nc.dram_tensor("out", [C, NQ], F32, kind="ExternalOutput")

    def split_h(ap):  # DRAM [256, w] -> [128, 2, w] (partition-first)
        return ap.rearrange("(h p) w -> p h w", p=128)

    with tile.TileContext(nc) as tc, ExitStack() as ctx:
        big = ctx.enter_context(tc.tile_pool(name="big", bufs=1))
        small = ctx.enter_context(tc.tile_pool(name="small", bufs=1))
        ppool = ctx.enter_context(tc.tile_pool(name="p", bufs=6))
        opool = ctx.enter_context(tc.tile_pool(name="o", bufs=2))
        dpool = ctx.enter_context(tc.tile_pool(name="d", bufs=2))
        spsum = ctx.enter_context(tc.tile_pool(name="spsum", bufs=2, space="PSUM"))
        apsum = ctx.enter_context(tc.tile_pool(name="apsum", bufs=4, space="PSUM"))
        dpsum = ctx.enter_context(tc.tile_pool(name="dpsum", bufs=1, space="PSUM"))

        # --- SBUF residents ---
        wq_sb = small.tile([128, 2, C], MM, tag="wq")
        wk_sb = small.tile([128, 2, C], MM, tag="wk")
        wv_sb = small.tile([128, 2, C], MM, tag="wv")
        bq_sb = small.tile([128, 2, 1], F32, tag="bq")
        bk_sb = small.tile([128, 2, 1], F32, tag="bk")
        x1_sb = big.tile([128, 2, NQ], MM, tag="x1")
        x2_sb = big.tile([128, 2, NK], MM, tag="x2")
        q_sb = big.tile([128, 2, NQ], MM, tag="q")
        k_sb = big.tile([128, 2, NK], MM, tag="k")
        v_sb = big.tile([128, NK_TILES, C], F16, tag="v")

        # --- DMA triggers, earliest; ordered by first consumption. Sync
        # queue carries the critical stream (weights + x-data) since its
        # preamble clears first; Activation queue (blocked ~1.3us longer by
        # the exp table load) carries the biases, needed slightly later.
        # Each trigger moves both 128-row halves (merged descriptor). ---
        nc.sync.dma_start(out=wk_sb[:], in_=split_h(wk[:, :]))
        nc.sync.dma_start(out=x2_sb[:, :, 0:XCHUNK], in_=split_h(x2[:, 0:XCHUNK]))
        nc.sync.dma_start(out=x1_sb[:, :, 0:CHUNK], in_=split_h(x1[:, 0:CHUNK]))
        nc.sync.dma_start(out=wq_sb[:], in_=split_h(wq[:, :]))
        nc.sync.dma_start(out=wv_sb[:], in_=split_h(wv[:, :]))
        for j in range(1, NJ):
            xs_ = slice(j * XCHUNK, (j + 1) * XCHUNK)
            nc.sync.dma_start(out=x2_sb[:, :, xs_], in_=split_h(x2[:, xs_]))
        nc.sync.dma_start(out=x1_sb[:, :, CHUNK:NQ], in_=split_h(x1[:, CHUNK:NQ]))

        nc.scalar.dma_start(out=bk_sb[:], in_=split_h(bk[:, :]))
        nc.scalar.dma_start(out=bq_sb[:], in_=split_h(bq[:, :]))
        # bv as a [1, 2, 128] f32r row for the bias-fold matmul of the
        # final chunk (bias enters as bv (x) den before normalization)
        bv_row = small.tile([1, 2, 128], MM, tag="bv_row")
        nc.scalar.dma_start(
            out=bv_row[:], in_=bv[:, :].rearrange("(h p) o -> o h p", p=128).bitcast(F32R)
        )

        ones_col_f32 = small.tile([128, 1], F32, tag="ones_col_f32")
        nc.vector.memset(ones_col_f32[:], 1.0)
        ones_col = small.tile([128, 1], MM, tag="ones_col")
        nc.vector.tensor_copy(ones_col[:], ones_col_f32[:])
        ones_row_f32 = small.tile([1, 128], F32, tag="ones_row_f32")
        nc.vector.memset(ones_row_f32[:], 1.0)
        ones_row = small.tile([1, 128], MM, tag="ones_row")
        nc.vector.tensor_copy(ones_row[:], ones_row_f32[:])
        # bv broadcast to all partitions: bias-fold matmul stationary
        # (acc_ct += bv_ct (x) den, so no per-half DVE bias add is needed)
        bvb_ps = spsum.tile([128, 2 * 128], F32, tag="s", name="bvb_ps")
        nc.tensor.matmul(
            bvb_ps[:], ones_row[:], bv_row[:].rearrange("o h p -> o (h p)"),
            start=True, stop=True,
        )
        bv_bcast = small.tile([128, 2 * 128], MM, tag="bv_bcast")
        nc.scalar.copy(bv_bcast[:], bvb_ps[:])

        # --- projection helpers ---
        def kproj(j):
            cs = slice(j * XCHUNK, (j + 1) * XCHUNK)
            for ct in range(2):
                kp = spsum.tile([128, XCHUNK], F32, tag="s", name="kp")
                cts = slice(ct * 128, (ct + 1) * 128)
                nc.tensor.matmul(kp[:], wk_sb[:, 0, cts], x2_sb[:, 0, cs], start=True, stop=False)
                nc.tensor.matmul(kp[:], wk_sb[:, 1, cts], x2_sb[:, 1, cs], start=False, stop=True)
                nc.vector.tensor_scalar_add(k_sb[:, ct, cs], kp[:], bk_sb[:, ct, :])

        def vproj(j):
            for t in range(j * (XCHUNK // 128), (j + 1) * (XCHUNK // 128)):
                ts = slice(t * 128, (t + 1) * 128)
                vp = spsum.tile([128, C], F32, tag="s", name="vp")
                nc.tensor.matmul(vp[:], x2_sb[:, 0, ts], wv_sb[:, 0, :], start=True, stop=False)
                nc.tensor.matmul(vp[:], x2_sb[:, 1, ts], wv_sb[:, 1, :], start=False, stop=True)
                nc.scalar.copy(v_sb[:, t, :], vp[:])

        def qproj(c0):
            cs = slice(c0 * CHUNK, (c0 + 1) * CHUNK)
            for ct in range(2):
                qp = spsum.tile([128, CHUNK], F32, tag="s", name="qp")
                cts = slice(ct * 128, (ct + 1) * 128)
                nc.tensor.matmul(qp[:], wq_sb[:, 0, cts], x1_sb[:, 0, cs], start=True, stop=False)
                nc.tensor.matmul(qp[:], wq_sb[:, 1, cts], x1_sb[:, 1, cs], start=False, stop=True)
                nc.vector.tensor_scalar_add(q_sb[:, ct, cs], qp[:], bq_sb[:, ct, :])

        # --- attention chunk state ---
        class ChunkState:
            def __init__(self, c0):
                self.c0 = c0
                self.cs = slice(c0 * CHUNK, (c0 + 1) * CHUNK)
                self.acc0 = apsum.tile([128, CHUNK], F32, tag="acc", name="acc0")
                self.acc1 = apsum.tile([128, CHUNK], F32, tag="acc", name="acc1")
                # P-sum split across Pool (even tiles) and DVE (odd) so
                # neither engine's serial accumulation chain gates the PE.
                self.psum_p = dpool.tile([128, CHUNK], F16, tag="psum_p", name="psum_p")
                self.psum_d = dpool.tile([128, CHUNK], F16, tag="psum_d", name="psum_d")
                self.p_tiles = {}

        def s_tile(st, t):
            ts = slice(t * 128, (t + 1) * 128)
            sp = spsum.tile([128, CHUNK], F32, tag="s", name="sp")
            nc.tensor.matmul(sp[:], k_sb[:, 0, ts], q_sb[:, 0, st.cs], start=True, stop=False)
            nc.tensor.matmul(sp[:], k_sb[:, 1, ts], q_sb[:, 1, st.cs], start=False, stop=True)
            p = ppool.tile([128, CHUNK], F16, tag="p", name="p")
            nc.scalar.activation(p[:], sp[:], mybir.ActivationFunctionType.Exp, scale=SCALE)
            st.p_tiles[t] = p

        def emit_pv(st, t):
            first = t == 0
            p = st.p_tiles.pop(t)
            # stop stays False on t=31: the bias-fold matmul closes the group
            nc.tensor.matmul(st.acc0[:], v_sb[:, t, 0:128], p[:], start=first, stop=False)
            nc.tensor.matmul(st.acc1[:], v_sb[:, t, 128:256], p[:], start=first, stop=False)
            if t == NK_TILES - 1:
                # last tile's P joins via the tree-balanced combine below
                st.p31 = p
                return
            eng, acc_ps = (nc.gpsimd, st.psum_p) if t % 2 == 0 else (nc.vector, st.psum_d)
            if t < 2:
                eng.tensor_copy(acc_ps[:], p[:])
            else:
                eng.tensor_add(acc_ps[:], acc_ps[:], p[:])
            if t == NK_TILES - 2:
                # evens(0..30) + odds(1..29) combine, off the critical path
                st.comb = dpool.tile([128, CHUNK], F32, tag="comb", name="comb")
                nc.gpsimd.tensor_add(st.comb[:], st.psum_p[:], st.psum_d[:])

        def flush_chunk(st):
            for t in range(NK_TILES - PIPE, NK_TILES):
                emit_pv(st, t)
            # P total = comb + p31; one short DVE link after the last exp
            st.acc_r = dpool.tile([128, CHUNK], MM, tag="acc_r", name="acc_r")
            nc.vector.tensor_add(st.acc_r[:], st.comb[:], st.p31[:])

        # --- softmax tails. tail_a: denominator + reciprocal. tail_b:
        # broadcast + normalize + bias + out DMA. Both run for chunk c
        # while chunk c+1's S/PV stream keeps the PE busy; `final` splits
        # the output halves across DVE and Pool to shorten the exposed
        # end-of-kernel chain. ---
        def tail_a(st):
            den = dpsum.tile([1, CHUNK], F32, tag="den", name="den")
            nc.tensor.matmul(den[:], ones_col[:], st.acc_r[:], start=True, stop=True)
            # bias fold: acc_ct += bv_ct (x) den == bv_bcast_ct^T @ acc_r;
            # closes the PV accumulation group (stop=True)
            nc.tensor.matmul(st.acc0[:], bv_bcast[:, 0:128], st.acc_r[:], start=False, stop=True)
            nc.tensor.matmul(st.acc1[:], bv_bcast[:, 128:256], st.acc_r[:], start=False, stop=True)
            recip_f32 = dpool.tile([1, CHUNK], F32, tag="recip_f32", name="recip_f32")
            nc.vector.reciprocal_approx_fast(out=recip_f32[:], in_=den[:])
            recip = dpool.tile([1, CHUNK], MM, tag="recip", name="recip")
            nc.vector.tensor_copy(recip[:], recip_f32[:])
            st.recip = recip

        def tail_b(st, final=False):
            bcast = dpsum.tile([128, CHUNK], F32, tag="bcast", name="bcast")
            nc.tensor.matmul(bcast[:], ones_row[:], st.recip[:], start=True, stop=True)
            bcast_sb = opool.tile([128, CHUNK], F32, tag="bcast_sb", name="bcast_sb")
            nc.scalar.copy(bcast_sb[:], bcast[:])
            if final:
                # bias already folded; separate tiles per half so neither
                # DMA waits on the other half's writer
                o0 = opool.tile([128, CHUNK], F32, tag="o_f0", name="o_f0")
                nc.vector.tensor_mul(o0[:], st.acc0[:], bcast_sb[:])
                nc.sync.dma_start(
                    out=split_h(out[:, st.cs])[:, 0:1, :],
                    in_=o0[:].rearrange("p (o w) -> p o w", o=1),
                )
                o1 = opool.tile([128, CHUNK], F32, tag="o_f1", name="o_f1")
                nc.vector.tensor_mul(o1[:], st.acc1[:], bcast_sb[:])
                nc.sync.dma_start(
                    out=split_h(out[:, st.cs])[:, 1:2, :],
                    in_=o1[:].rearrange("p (o w) -> p o w", o=1),
                )
            else:
                o2 = opool.tile([128, 2, CHUNK], F32, tag="o2", name="o2")
                for ct, acc in ((0, st.acc0), (1, st.acc1)):
                    nc.vector.tensor_mul(o2[:, ct, :], acc[:], bcast_sb[:])
                nc.sync.dma_start(out=split_h(out[:, st.cs]), in_=o2[:])

        # ================= program =================
        # Phase A: stream projections behind DMA; interleave chunk-0
        # attention one x2-chunk behind the projections.
        kproj(0)
        qproj(0)
        vproj(0)
        st = ChunkState(0)
        for j in range(1, NJ):
            for t in range((j - 1) * 4, j * 4):
                s_tile(st, t)
                if t >= PIPE:
                    emit_pv(st, t - PIPE)
            kproj(j)
            vproj(j)
        for t in range((NJ - 1) * 4, NK_TILES):
            s_tile(st, t)
            if t >= PIPE:
                emit_pv(st, t - PIPE)
        flush_chunk(st)
        for c0 in range(1, NQ_CHUNKS):
            qproj(c0)

        # Phase B: chunks 1..3; previous chunk's tail is woven into this
        # chunk's S stream (tail_a after tile 1, tail_b after tile 5).
        prev = st
        for c0 in range(1, NQ_CHUNKS):
            st = ChunkState(c0)
            for t in range(NK_TILES):
                s_tile(st, t)
                if t == 4 and prev is not None:
                    tail_a(prev)
                if t == 8 and prev is not None:
                    tail_b(prev)
                    prev = None
                if t >= PIPE and t - PIPE < NK_TILES - PIPE:
                    emit_pv(st, t - PIPE)
            flush_chunk(st)
            prev = st

        # final chunk's tail is exposed: shortest possible chain
        tail_a(prev)
        tail_b(prev, final=True)

    nc.compile()
    return nc


def core_inputs(inputs, core):
    """Slice full-problem inputs for one core (numpy)."""
    b, h = core // 2, core % 2
    x1r = np.asarray(inputs["x1"], dtype=np.float32).reshape(B, C, N)
    x2r = np.asarray(inputs["x2"], dtype=np.float32).reshape(B, C, N)
    return {
        "x1c": np.ascontiguousarray(x1r[b][:, h * NQ : (h + 1) * NQ]),
        "x2c": np.ascontiguousarray(x2r[b]),
        "wqT": np.ascontiguousarray(np.asarray(inputs["Wq"], dtype=np.float32).T),
        "wkT": np.ascontiguousarray(np.asarray(inputs["Wk"], dtype=np.float32).T),
        "wvT": np.ascontiguousarray(np.asarray(inputs["Wv"], dtype=np.float32).T),
        "bq": np.asarray(inputs["bq"], dtype=np.float32).reshape(C, 1).copy(),
        "bk": np.asarray(inputs["bk"], dtype=np.float32).reshape(C, 1).copy(),
        "bv": np.asarray(inputs["bv"], dtype=np.float32).reshape(C, 1).copy(),
    }


_NC_CACHE = {}


def get_nc():
    if "nc" not in _NC_CACHE:
        _NC_CACHE["nc"] = build_nc()
    return _NC_CACHE["nc"]


def kernel(**inputs) -> np.ndarray:
    """Full-problem entry point: full inputs in, full [4,256,64,64] f32 out."""
    nc = get_nc()
    in_maps = [core_inputs(inputs, core) for core in range(8)]
    res = run_bass_kernel_spmd(nc, in_maps, list(range(8)))
    full = np.zeros((B, C, N), np.float32)
    for core in range(8):
        b, h = core // 2, core % 2
        full[b][:, h * NQ : (h + 1) * NQ] = res.results[core]["out"]
    return full.reshape(B, C, H, W)
